# revision 3
# baseline (speedup 1.0000x reference)
"""Trainium2 Bass kernel for nn_EvolutionBlock (moe_routing) — v3.

v2 (sparse top-2 expert dispatch) + fully-fused SSM branch.

SSM fusion: ssm = sW_out @ causal_conv(sW_in @ x) is linear end-to-end,
so it equals a single 4-tap causal conv on x with host-precomputed
weights W''_k = sW_out @ W_k @ sW_in (fp64) and a folded bias
sW_out @ (sum_k W_k @ sb_in + sb_conv) + sb_out. The left zero-padding
of the conv (reference pads post-sW_in states with zeros, which do NOT
carry sb_in) is made exact with a tiny per-core correction matrix
added to the first 3 tokens of batch-initial blocks. This removes the
hT and sW_out phases (~60us of PE) at identical conv cost.

MoE dispatch: core c owns expert e=c. Host gathers x columns of every
token whose fp64 top-2 router choice includes e (verified to match the
fp32 reference selection) into a padded slot buffer; the device
recomputes the 11 router logits per slot (rows permuted so row0 = own
expert), forms s = sigmoid(l_e - max_{j!=e} l_j) * bw2 on-chip, runs
the expert FFN, returns s*(FFN(x)+eb2) per slot; host scatter-adds.
"""

import numpy as np
import ml_dtypes

import concourse.bass as bass
import concourse.tile as tile
from concourse import bacc, mybir
from concourse import bass_utils

F32 = mybir.dt.float32
BF16 = mybir.dt.bfloat16
AF = mybir.ActivationFunctionType
ALU = mybir.AluOpType
AX = mybir.AxisListType
BF = ml_dtypes.bfloat16

# Problem constants
B, T, D = 4, 2048, 1024
HD = 4096          # dense hidden (fc1 out = 2*HD)
S, KC_ = 1024, 4   # ssm state, conv kernel
E, HE = 8, 512     # experts, expert hidden
NCORE = 8
TOKENS = B * T
TOK = TOKENS // NCORE   # tokens per core
HALO = 3
DC = D // 128           # 8 d-chunks
CAP = 2176              # dispatch slot capacity (multiple of 128)


def _coltiles(n, w=512):
    out = []
    c = 0
    while c < n:
        out.append((c, min(w, n - c)))
        c += w
    return out


def build_program(ntok=TOK, cap=CAP):
    nt = ntok + HALO
    nc = bacc.Bacc("TRN2", target_bir_lowering=False, debug=False,
                   num_devices=NCORE)

    def din(name, shape, dt):
        return nc.dram_tensor(name, list(shape), dt, kind="ExternalInput").ap()

    xs_d = din("x_s", [128, DC * nt], BF16)
    xd_d = din("xd_s", [128, DC * cap], BF16)
    wr3_d = din("w_r3", [128, DC * 3], BF16)
    rb3_d = din("rb3", [3, 1], F32)
    wr11_d = din("w_r11", [128, DC * 11], BF16)
    rb11_d = din("rb11", [11, 1], F32)
    id11_d = din("ident11", [11, 11], F32)
    ident_d = din("ident", [128, 128], BF16)
    ones_d = din("ones1", [1, 128], BF16)
    ones3_d = din("ones3", [3, 1], F32)
    sel1_d = din("sel1", [3, 128], F32)
    wconv_d = din("w_conv", [128, 8 * 32 * 128], BF16)
    bconv_d = din("b_conv", [128, 8], F32)
    corr3_d = din("corr3", [3, 1024], BF16)
    eye3_d = din("eye3", [3, 512], BF16)
    b2a_d = din("b2a", [1, 1024], BF16)
    we1_d = din("w_e1", [128, 64 * 128], BF16)
    be1a_d = din("b_e1a", [128, 4], F32)
    be1b_d = din("b_e1b", [128, 4], F32)
    we2_d = din("w_e2", [128, 32 * 128], BF16)
    eb2c_d = din("eb2c", [128, 8], F32)
    wd1a_d = din("w_d1a", [128, 256 * 128], BF16)
    wd1b_d = din("w_d1b", [128, 256 * 128], BF16)
    bd1a_d = din("b_d1a", [128, 32], F32)
    bd1b_d = din("b_d1b", [128, 32], F32)
    wd2_d = din("w_d2", [128, 256 * 128], BF16)

    out_d = nc.dram_tensor("outT", [128, DC * ntok], F32,
                           kind="ExternalOutput").ap()
    outd_d = nc.dram_tensor("outD", [128, DC * cap], F32,
                            kind="ExternalOutput").ap()

    cts = _coltiles(ntok)
    cts_c = _coltiles(cap)
    ncap = cap // 128

    with tile.TileContext(nc) as tc:
        live = []

        def P(name, bufs, space="SBUF", side="left"):
            p = tc.alloc_tile_pool(name=name, bufs=bufs, space=space,
                                   side=side)
            live.append(p)
            return p

        def rel(*ps):
            for p in ps:
                live.remove(p)
                p.release()

        constp = P("constp", 1)
        xp = P("xp", 1)
        dw = P("dw", 3, side="right")
        e1w = P("e1w", 1, side="right")
        e2w = P("e2w", 1, side="right")
        xdp = P("xdp", 1, side="right")

        x_s = xp.tile([128, DC * nt], BF16)
        w_r3 = constp.tile([128, DC * 3], BF16)
        nc.sync.dma_start(w_r3[:], wr3_d[:])
        for kc in range(DC):
            nc.sync.dma_start(x_s[:, kc * nt:(kc + 1) * nt],
                              xs_d[:, kc * nt:(kc + 1) * nt])

        r3t = P("r3t", 1)
        # conv weight slabs: prefetch 2 deep; rest stream in-loop
        cwp = P("cwp", 2)
        wcvs = []
        for oc in range(2):
            t = cwp.tile([128, 32 * 128], BF16, tag="wcv", name="wcv")
            nc.sync.dma_start(
                t[:], wconv_d[:, oc * 32 * 128:(oc + 1) * 32 * 128])
            wcvs.append(t)

        xd_s = xdp.tile([128, DC * cap], BF16)
        we1 = e1w.tile([128, 64 * 128], BF16)
        we2 = e2w.tile([128, 32 * 128], BF16)

        ident = constp.tile([128, 128], BF16)
        nc.sync.dma_start(ident[:], ident_d[:])
        ident11 = constp.tile([11, 11], F32)
        nc.sync.dma_start(ident11[:], id11_d[:])
        ones1 = constp.tile([1, 128], BF16)
        nc.sync.dma_start(ones1[:], ones_d[:])
        ones3 = constp.tile([3, 1], F32)
        nc.sync.dma_start(ones3[:], ones3_d[:])
        sel1 = constp.tile([3, 128], F32)
        nc.sync.dma_start(sel1[:], sel1_d[:])
        rb3 = constp.tile([3, 1], F32)
        nc.sync.dma_start(rb3[:], rb3_d[:])
        w_r11 = constp.tile([128, DC * 11], BF16)
        nc.sync.dma_start(w_r11[:], wr11_d[:])
        rb11 = constp.tile([11, 1], F32)
        nc.sync.dma_start(rb11[:], rb11_d[:])
        b_conv = constp.tile([128, 8], F32)
        nc.sync.dma_start(b_conv[:], bconv_d[:])
        corr3 = constp.tile([3, 1024], BF16)
        nc.sync.dma_start(corr3[:], corr3_d[:])
        eye3 = constp.tile([3, 512], BF16)
        nc.sync.dma_start(eye3[:], eye3_d[:])
        b2a = constp.tile([1, 1024], BF16)
        nc.sync.dma_start(b2a[:], b2a_d[:])
        b_e1a = constp.tile([128, 4], F32)
        nc.sync.dma_start(b_e1a[:], be1a_d[:])
        b_e1b = constp.tile([128, 4], F32)
        nc.sync.dma_start(b_e1b[:], be1b_d[:])
        eb2c = constp.tile([128, 8], F32)
        nc.sync.dma_start(eb2c[:], eb2c_d[:])
        b_d1a = constp.tile([128, 32], F32)
        nc.sync.dma_start(b_d1a[:], bd1a_d[:])
        b_d1b = constp.tile([128, 32], F32)
        nc.sync.dma_start(b_d1b[:], bd1b_d[:])

        s_row = constp.tile([1, cap], BF16)
        wbm = constp.tile([128, cap], BF16)
        wb0 = constp.tile([128, ntok], BF16)
        wb1 = constp.tile([128, ntok], BF16)
        bw0r = constp.tile([1, ntok], BF16)
        out_acc = constp.tile([128, DC * ntok], F32)

        # ============ Phase R3: owner branch router ============
        # bw = softmax(x@rW.T + rb); need rows 0 (dense) and 1 (ssm).
        r3ps = P("r3ps", 1, "PSUM", side="right")
        lg3 = r3t.tile([3, ntok], F32)
        for (c0, cw) in cts:
            ps = r3ps.tile([3, 512], F32, tag="r3psum", name="r3psum")
            for kc in range(DC):
                nc.tensor.matmul(
                    ps[:, :cw], w_r3[:, kc * 3:(kc + 1) * 3],
                    x_s[:, kc * nt + HALO + c0:kc * nt + HALO + c0 + cw],
                    start=(kc == 0), stop=(kc == DC - 1))
            nc.scalar.activation(lg3[:, c0:c0 + cw], ps[:, :cw],
                                 AF.Identity, bias=rb3[:, 0:1])
        e3f = r3t.tile([3, ntok], F32)
        nc.scalar.activation(e3f[:], lg3[:], AF.Exp)
        ssum = r3t.tile([1, ntok], F32)
        for (c0, cw) in cts:
            ps1 = r3ps.tile([1, 512], F32, tag="r3sum", name="r3sum")
            nc.tensor.matmul(ps1[:, :cw], ones3[:], e3f[:, c0:c0 + cw],
                             start=True, stop=True)
            nc.scalar.copy(ssum[:, c0:c0 + cw], ps1[:, :cw])
        rcp = r3t.tile([1, ntok], F32)
        nc.vector.reciprocal(rcp[:], ssum[:])
        nc.vector.tensor_mul(bw0r[:], e3f[0:1, :], rcp[:])
        rcq = r3t.tile([1, ntok], BF16)
        nc.vector.tensor_copy(rcq[:], rcp[:])
        eb1t = r3t.tile([128, ntok], BF16)
        rcpb = r3t.tile([128, ntok], BF16)
        for (c0, cw) in cts:
            pb = r3ps.tile([128, 512], F32, tag="r3b", name="r3b")
            nc.tensor.matmul(pb[:, :cw], ones1[:], bw0r[:, c0:c0 + cw],
                             start=True, stop=True)
            nc.scalar.copy(wb0[:, c0:c0 + cw], pb[:, :cw])
            pb2 = r3ps.tile([128, 512], F32, tag="r3b", name="r3b")
            nc.tensor.matmul(pb2[:, :cw], sel1[:], e3f[:, c0:c0 + cw],
                             start=True, stop=True)
            nc.scalar.copy(eb1t[:, c0:c0 + cw], pb2[:, :cw])
            pb3 = r3ps.tile([128, 512], F32, tag="r3b", name="r3b")
            nc.tensor.matmul(pb3[:, :cw], ones1[:], rcq[:, c0:c0 + cw],
                             start=True, stop=True)
            nc.scalar.copy(rcpb[:, c0:c0 + cw], pb3[:, :cw])
        nc.vector.tensor_mul(wb1[:], eb1t[:], rcpb[:])
        rel(r3ps)

        # ========= Phase C: fused SSM conv -> out_acc (init) =========
        # out_acc = wb1 * (conv''(x) + bias_ssm [+ corr3 on first 3 cols])
        cps = P("cps", 3, "PSUM")
        for oc in range(DC):
            wcv = wcvs[oc]
            if oc + 2 < DC:
                t = cwp.tile([128, 32 * 128], BF16, tag="wcv", name="wcv")
                nc.sync.dma_start(
                    t[:], wconv_d[:, (oc + 2) * 32 * 128:
                                  (oc + 3) * 32 * 128])
                wcvs.append(t)
            if oc == 4:
                nc.sync.dma_start(xd_s[:], xd_d[:])
            if oc == 5:
                nc.sync.dma_start(we1[:], we1_d[:])
            if oc == 6:
                nc.sync.dma_start(we2[:], we2_d[:])
            for (c0, cw) in cts:
                ps = cps.tile([128, 512], F32, tag="cpsum", name="cpsum")
                has_corr = (c0 == 0)
                n = 0
                for k in range(KC_):
                    for ic in range(DC):
                        nc.tensor.matmul(
                            ps[:, :cw],
                            wcv[:, (k * 8 + ic) * 128:(k * 8 + ic + 1) * 128],
                            x_s[:, ic * nt + c0 + k:ic * nt + c0 + k + cw],
                            start=(n == 0),
                            stop=(not has_corr and n == KC_ * DC - 1))
                        n += 1
                if has_corr:
                    nc.tensor.matmul(
                        ps[:, :cw], corr3[:, oc * 128:(oc + 1) * 128],
                        eye3[:, :cw], start=False, stop=True)
                nc.vector.scalar_tensor_tensor(
                    out=out_acc[:, oc * ntok + c0:oc * ntok + c0 + cw],
                    in0=ps[:, :cw], scalar=b_conv[:, oc:oc + 1],
                    in1=wb1[:, c0:c0 + cw], op0=ALU.add, op1=ALU.mult)

        # ============ Phase RD: slot router ============
        # rows: 0 = own expert logit, 1-7 = other experts, 8-10 = branch
        rdp = P("rdp", 1, side="right")
        rds = P("rds", 4, side="right")
        rdps = P("rdps", 1, "PSUM", side="right")
        lg11 = rdp.tile([11, cap], F32)
        for (c0, cw) in cts_c:
            ps = rdps.tile([11, 512], F32, tag="rdpsum", name="rdpsum")
            for kc in range(DC):
                nc.tensor.matmul(
                    ps[:, :cw], w_r11[:, kc * 11:(kc + 1) * 11],
                    xd_s[:, kc * cap + c0:kc * cap + c0 + cw],
                    start=(kc == 0), stop=(kc == DC - 1))
            nc.scalar.activation(lg11[:, c0:c0 + cw], ps[:, :cw],
                                 AF.Identity, bias=rb11[:, 0:1])
        for tcn in range(ncap):
            pst = rdps.tile([128, 11], F32, tag="rdt", name="rdt")
            nc.tensor.transpose(pst[:], lg11[:, tcn * 128:(tcn + 1) * 128],
                                ident11[:])
            rsb = rds.tile([128, 11], F32, tag="rsb", name="rsb")
            nc.scalar.copy(rsb[:], pst[:])
            mo = rds.tile([128, 1], F32, tag="mo", name="mo")
            nc.vector.reduce_max(mo[:], rsb[:, 1:8], axis=AX.X)
            dv = rds.tile([128, 1], F32, tag="dv", name="dv")
            nc.vector.tensor_sub(dv[:], rsb[:, 0:1], mo[:])
            wsig = rds.tile([128, 1], F32, tag="wsig", name="wsig")
            nc.scalar.activation(wsig[:], dv[:], AF.Sigmoid)
            e3 = rds.tile([128, 3], F32, tag="e3", name="e3")
            nc.scalar.activation(e3[:], rsb[:, 8:11], AF.Exp)
            s3 = rds.tile([128, 1], F32, tag="s3", name="s3")
            nc.vector.reduce_sum(s3[:], e3[:], axis=AX.X)
            rpc = rds.tile([128, 1], F32, tag="rpc", name="rpc")
            nc.vector.reciprocal(rpc[:], s3[:])
            bw2 = rds.tile([128, 1], F32, tag="bw2", name="bw2")
            nc.vector.tensor_scalar(out=bw2[:], in0=e3[:, 2:3], scalar1=rpc[:],
                                    scalar2=None, op0=ALU.mult)
            sv = rds.tile([128, 1], BF16, tag="sv", name="sv")
            nc.vector.tensor_mul(sv[:], wsig[:], bw2[:])
            pr = rdps.tile([1, 128], BF16, tag="rdr", name="rdr")
            nc.tensor.transpose(pr[:], sv[:], ident[:])
            nc.scalar.copy(s_row[:, tcn * 128:(tcn + 1) * 128], pr[:])
        for (c0, cw) in cts_c:
            pb = rdps.tile([128, 512], F32, tag="rdb", name="rdb")
            nc.tensor.matmul(pb[:, :cw], ones1[:], s_row[:, c0:c0 + cw],
                             start=True, stop=True)
            nc.scalar.copy(wbm[:, c0:c0 + cw], pb[:, :cw])
        rel(rds, rdp, rdps, cps, cwp, r3t)

        # ============ Phase MF1: expert fc1 + swiglu * scale ============
        gp = P("gp", 1)
        m1t = P("m1t", 4)
        m1ps = P("m1ps", 2, "PSUM")
        g_s = gp.tile([128, 4 * cap], BF16, name="g_s")
        wd1s = []
        for grp in range(2):
            t = dw.tile([128, 32 * 128], BF16, tag="wd1", name="wda")
            nc.sync.dma_start(
                t[:], wd1a_d[:, grp * 32 * 128:(grp + 1) * 32 * 128])
            wd1s.append(t)
        for (c0, cw) in cts_c:
            for m in range(4):
                psa = m1ps.tile([128, 512], F32, tag="psa", name="psa")
                psb = m1ps.tile([128, 512], F32, tag="psb", name="psb")
                for half, pst_ in ((0, psa), (1, psb)):
                    bi = (m * 2 + half) * 8
                    for kc in range(DC):
                        nc.tensor.matmul(
                            pst_[:, :cw],
                            we1[:, (bi + kc) * 128:(bi + kc + 1) * 128],
                            xd_s[:, kc * cap + c0:kc * cap + c0 + cw],
                            start=(kc == 0), stop=(kc == DC - 1))
                sg = m1t.tile([128, 512], BF16, tag="sg", name="sg")
                nc.scalar.activation(sg[:, :cw], psa[:, :cw], AF.Sigmoid,
                                     bias=b_e1a[:, m:m + 1])
                sa = m1t.tile([128, 512], BF16, tag="sa", name="sa")
                nc.vector.scalar_tensor_tensor(
                    out=sa[:, :cw], in0=psa[:, :cw],
                    scalar=b_e1a[:, m:m + 1],
                    in1=sg[:, :cw], op0=ALU.add, op1=ALU.mult)
                sa2 = m1t.tile([128, 512], BF16, tag="sa2", name="sa2")
                nc.vector.tensor_mul(sa2[:, :cw], sa[:, :cw],
                                     wbm[:, c0:c0 + cw])
                nc.vector.scalar_tensor_tensor(
                    out=g_s[:, m * cap + c0:m * cap + c0 + cw],
                    in0=psb[:, :cw], scalar=b_e1b[:, m:m + 1],
                    in1=sa2[:, :cw], op0=ALU.add, op1=ALU.mult)
        rel(m1t, xdp)

        # ============ Phase MF2: expert fc2 -> outD ============
        odp = P("odp", 3)
        m2ps = P("m2ps", 3, "PSUM", side="right")
        for mc in range(DC):
            for (c0, cw) in cts_c:
                ps = m2ps.tile([128, 512], F32, tag="m2psum", name="m2psum")
                for kc in range(4):
                    nc.tensor.matmul(
                        ps[:, :cw],
                        we2[:, (mc * 4 + kc) * 128:(mc * 4 + kc + 1) * 128],
                        g_s[:, kc * cap + c0:kc * cap + c0 + cw],
                        start=(kc == 0), stop=(kc == 3))
                od = odp.tile([128, 512], F32, tag="od", name="od")
                nc.vector.scalar_tensor_tensor(
                    out=od[:, :cw], in0=wbm[:, c0:c0 + cw],
                    scalar=eb2c[:, mc:mc + 1], in1=ps[:, :cw],
                    op0=ALU.mult, op1=ALU.add)
                nc.sync.dma_start(
                    outd_d[:, mc * cap + c0:mc * cap + c0 + cw],
                    od[:, :cw])
        rel(m1ps)
        rel(odp, gp, m2ps)

        # ================= Phase D: dense (split by fc2 h-half) =========
        sap = P("sap", 1, side="right")
        dt_ = P("dt", 2, side="right")
        d2w = P("d2w", 3, side="right")
        dps = P("dps", 2, "PSUM")
        d2ps = P("d2ps", 2, "PSUM")
        sa_s = sap.tile([128, 16 * ntok], BF16)
        wd1sa = {}
        for grp in range(2):
            t = dw.tile([128, 32 * 128], BF16, tag="wd1", name="wda")
            nc.sync.dma_start(
                t[:], wd1a_d[:, grp * 32 * 128:(grp + 1) * 32 * 128])
            wd1sa[grp] = t
        for hp_ in range(2):
            g0 = hp_ * 4
            # ---- fc1 a-half for this h ----
            for grp in range(g0, g0 + 4):
                wda = wd1sa.pop(grp)
                if grp + 2 < 8:
                    t = dw.tile([128, 32 * 128], BF16, tag="wd1", name="wda")
                    nc.sync.dma_start(
                        t[:], wd1a_d[:, (grp + 2) * 32 * 128:
                                      (grp + 3) * 32 * 128])
                    wd1sa[grp + 2] = t
                if grp == g0 + 3:
                    # prefetch this h's first b-slabs
                    for bg in range(g0, g0 + 2):
                        t = dw.tile([128, 32 * 128], BF16, tag="wd1",
                                    name="wdb")
                        nc.sync.dma_start(
                            t[:], wd1b_d[:, bg * 32 * 128:
                                          (bg + 1) * 32 * 128])
                        wd1sa[('b', bg)] = t
                for mcl in range(4):
                    mc = grp * 4 + mcl
                    ml = mc - hp_ * 16
                    for (c0, cw) in cts:
                        psa = dps.tile([128, 512], F32, tag="dpsa",
                                       name="dpsa")
                        for kc in range(DC):
                            nc.tensor.matmul(
                                psa[:, :cw],
                                wda[:, (mcl * 8 + kc) * 128:
                                    (mcl * 8 + kc + 1) * 128],
                                x_s[:, kc * nt + HALO + c0:
                                    kc * nt + HALO + c0 + cw],
                                start=(kc == 0), stop=(kc == DC - 1))
                        sg = dt_.tile([128, 512], BF16, tag="sg", name="sg")
                        nc.scalar.activation(sg[:, :cw], psa[:, :cw],
                                             AF.Sigmoid,
                                             bias=b_d1a[:, mc:mc + 1])
                        nc.vector.scalar_tensor_tensor(
                            out=sa_s[:, ml * ntok + c0:ml * ntok + c0 + cw],
                            in0=psa[:, :cw], scalar=b_d1a[:, mc:mc + 1],
                            in1=sg[:, :cw], op0=ALU.add, op1=ALU.mult)
            # ---- fc1 b-half for this h ----
            for grp in range(g0, g0 + 4):
                wdb = wd1sa.pop(('b', grp))
                if grp + 2 < g0 + 4:
                    t = dw.tile([128, 32 * 128], BF16, tag="wd1", name="wdb")
                    nc.sync.dma_start(
                        t[:], wd1b_d[:, (grp + 2) * 32 * 128:
                                      (grp + 3) * 32 * 128])
                    wd1sa[('b', grp + 2)] = t
                if grp == g0 + 3:
                    # prefetch this h's first fc2 slab
                    t = d2w.tile([128, 16 * 128], BF16, tag="wd2",
                                 name="wd2")
                    nc.sync.dma_start(
                        t[:], wd2_d[:, (hp_ * 8) * 16 * 128:
                                      (hp_ * 8 + 1) * 16 * 128])
                    wd1sa[('c', 0)] = t
                for mcl in range(4):
                    mc = grp * 4 + mcl
                    ml = mc - hp_ * 16
                    for (c0, cw) in cts:
                        psb = dps.tile([128, 512], F32, tag="dpsb",
                                       name="dpsb")
                        for kc in range(DC):
                            nc.tensor.matmul(
                                psb[:, :cw],
                                wdb[:, (mcl * 8 + kc) * 128:
                                    (mcl * 8 + kc + 1) * 128],
                                x_s[:, kc * nt + HALO + c0:
                                    kc * nt + HALO + c0 + cw],
                                start=(kc == 0), stop=(kc == DC - 1))
                        hb = dt_.tile([128, 512], BF16, tag="hb", name="hb")
                        nc.scalar.activation(hb[:, :cw], psb[:, :cw],
                                             AF.Identity,
                                             bias=b_d1b[:, mc:mc + 1])
                        hb2 = dt_.tile([128, 512], BF16, tag="hb2",
                                       name="hb2")
                        nc.vector.tensor_mul(hb2[:, :cw], hb[:, :cw],
                                             wb0[:, c0:c0 + cw])
                        nc.vector.tensor_mul(
                            sa_s[:, ml * ntok + c0:ml * ntok + c0 + cw],
                            sa_s[:, ml * ntok + c0:ml * ntok + c0 + cw],
                            hb2[:, :cw])
            # ---- fc2 for this h (+ bw0*d2b once, on the last h) ----
            for mc in range(DC):
                wd2 = wd1sa.pop(('c', mc))
                if mc + 1 < DC:
                    t = d2w.tile([128, 16 * 128], BF16, tag="wd2",
                                 name="wd2")
                    nc.sync.dma_start(
                        t[:], wd2_d[:, (hp_ * 8 + mc + 1) * 16 * 128:
                                      (hp_ * 8 + mc + 2) * 16 * 128])
                    wd1sa[('c', mc + 1)] = t
                for (c0, cw) in cts:
                    ps = d2ps.tile([128, 512], F32, tag="d2psum",
                                   name="d2psum")
                    for kc in range(16):
                        nc.tensor.matmul(
                            ps[:, :cw], wd2[:, kc * 128:(kc + 1) * 128],
                            sa_s[:, kc * ntok + c0:kc * ntok + c0 + cw],
                            start=(kc == 0), stop=(hp_ == 0 and kc == 15))
                    if hp_ == 1:
                        nc.tensor.matmul(ps[:, :cw],
                                         b2a[:, mc * 128:(mc + 1) * 128],
                                         bw0r[:, c0:c0 + cw],
                                         start=False, stop=True)
                    nc.vector.tensor_add(
                        out_acc[:, mc * ntok + c0:mc * ntok + c0 + cw],
                        out_acc[:, mc * ntok + c0:mc * ntok + c0 + cw],
                        ps[:, :cw])
                if hp_ == 1:
                    for (c0, cw) in cts:
                        nc.sync.dma_start(
                            out_d[:, mc * ntok + c0:mc * ntok + c0 + cw],
                            out_acc[:, mc * ntok + c0:mc * ntok + c0 + cw])
        for p in reversed(live):
            p.release()

    nc.compile()
    return nc


# ---------------- host-side packing ----------------

def _pack_mk(WT, kcn, mcn):
    """WT [K, M] -> [128, mcn*kcn*128] with block idx = mc*kcn+kc."""
    return np.ascontiguousarray(
        WT.reshape(kcn, 128, mcn, 128).transpose(1, 2, 0, 3)
        .reshape(128, mcn * kcn * 128))


def _featmajor(xt, ncols):
    """xt [1024, ncols] -> [128, 8*ncols] (kc-blocks along columns)."""
    return np.ascontiguousarray(
        xt.reshape(DC, 128, ncols).transpose(1, 0, 2).reshape(128, DC * ncols))


def _bias_cols(b, n):
    """b [n*128] -> [128, n] with col i = b[i*128:(i+1)*128]."""
    return np.ascontiguousarray(
        np.asarray(b, np.float32).reshape(n, 128).T).astype(np.float32)


def pack_weights(rW, rb, d1W, d1b, d2W, d2b, sW_in, sb_in, sW_conv, sb_conv,
                 sW_out, sb_out, mW, mb, eW1, eb1, eW2, eb2):
    f32 = np.float32
    f64 = np.float64
    w = {}
    w["w_r3"] = _featmajor(rW.T.astype(BF), 3)
    w["rb3"] = rb[:, None].astype(f32)
    w["ident11"] = np.eye(11, dtype=f32)
    w["ident"] = np.eye(128, dtype=BF)
    w["ones1"] = np.ones((1, 128), dtype=BF)
    w["ones3"] = np.ones((3, 1), dtype=f32)
    sel1 = np.zeros((3, 128), f32)
    sel1[1, :] = 1.0
    w["sel1"] = sel1

    # ---- fused SSM conv: W''_k = sW_out @ W_k @ sW_in ----
    si = sW_in.astype(f64)
    so = sW_out.astype(f64)
    cv = sW_conv.astype(f64)                 # [S(out), S(in), K]
    Wpp = [so @ cv[:, :, k] @ si for k in range(KC_)]       # [D, D] out,in
    A3 = np.stack([Wk.T for Wk in Wpp])                     # [K, in, out]
    w["w_conv"] = np.ascontiguousarray(
        A3.reshape(4, 8, 128, 8, 128).transpose(2, 3, 0, 1, 4)
        .reshape(128, 8 * 32 * 128).astype(f32)).astype(BF)
    hb = sum(cv[:, :, k] @ sb_in.astype(f64) for k in range(KC_)) \
        + sb_conv.astype(f64)
    bias_ssm = so @ hb + sb_out.astype(f64)                 # [D]
    w["b_conv"] = _bias_cols(bias_ssm.astype(f32), 8)
    zS = np.zeros(S, f64)
    corr = np.stack([
        -(so @ sum((cv[:, :, k] @ sb_in.astype(f64) for k in range(3 - t)),
                   zS))
        for t in range(3)])                                 # [3, D]
    w["_corr3"] = corr.astype(f32).astype(BF)
    eye3 = np.zeros((3, 512), f32)
    eye3[0, 0] = eye3[1, 1] = eye3[2, 2] = 1.0
    w["eye3"] = eye3.astype(BF)

    w["b2a"] = d2b[None, :].astype(BF)
    w["w_d1a"] = _pack_mk(d1W[:HD].T.astype(BF), 8, 32)
    w["w_d1b"] = _pack_mk(d1W[HD:].T.astype(BF), 8, 32)
    w["b_d1a"] = _bias_cols(d1b[:HD], 32)
    w["b_d1b"] = _bias_cols(d1b[HD:], 32)
    # d2: block idx = h*128 + mc*16 + kcl, kg = h*16+kcl
    T4 = d2W.T.astype(BF).reshape(2, 16, 128, 8, 128)       # h,kcl,p,mc,c
    w["w_d2"] = np.ascontiguousarray(
        T4.transpose(2, 0, 3, 1, 4).reshape(128, 256 * 128))

    # ---- per-core (expert-parallel) tensors, stashed under "_" keys ----
    perm = {}
    e1l, e2l, r11l, rb11l, be1al, be1bl, eb2cl = [], [], [], [], [], [], []
    for e in range(E):
        others = [j for j in range(E) if j != e]
        R11 = np.concatenate([mW[e:e + 1], mW[others], rW], axis=0)  # [11,D]
        r11l.append(_featmajor(R11.T.astype(BF), 11))
        rb11l.append(np.concatenate(
            [mb[e:e + 1], mb[others], rb])[:, None].astype(f32))
        Tt = eW1[e].T.astype(BF).reshape(8, 128, 8, 128)   # kc,p,mcol,c
        blocks = [Tt[kc, :, half * 4 + m, :]
                  for m in range(4) for half in range(2) for kc in range(8)]
        e1l.append(np.ascontiguousarray(np.concatenate(blocks, axis=1)))
        T2 = eW2[e].T.astype(BF).reshape(4, 128, 8, 128)   # kc,p,mc,c
        blocks2 = [T2[kc, :, mc, :] for mc in range(8) for kc in range(4)]
        e2l.append(np.ascontiguousarray(np.concatenate(blocks2, axis=1)))
        be1al.append(_bias_cols(eb1[e, :HE], 4))
        be1bl.append(_bias_cols(eb1[e, HE:], 4))
        eb2cl.append(_bias_cols(eb2[e], 8))
    perm["w_e1"] = e1l
    perm["w_e2"] = e2l
    perm["w_r11"] = r11l
    perm["rb11"] = rb11l
    perm["b_e1a"] = be1al
    perm["b_e1b"] = be1bl
    perm["eb2c"] = eb2cl
    w["_percore"] = perm
    w["_mW"] = np.asarray(mW, np.float64)
    w["_mb"] = np.asarray(mb, np.float64)
    return w


def dispatch_indices(x, mW, mb):
    """Top-2 expert index sets per expert, matching reference top_k."""
    xt = np.asarray(x, np.float64).reshape(-1, D)
    L = xt @ mW.T + mb                                   # [tokens, E]
    top1 = L.argmax(1)
    L2 = L.copy()
    L2[np.arange(L.shape[0]), top1] = -np.inf
    top2 = L2.argmax(1)
    idx = [np.where((top1 == e) | (top2 == e))[0] for e in range(E)]
    return idx


def make_in_maps(x, weights, ntok=TOK, ncores=NCORE, cap=CAP):
    """x [B,T,D] fp32 -> (list of per-core in_maps, dispatch idx lists)."""
    xt = np.asarray(x, np.float32).reshape(-1, D).T           # [D, tokens]
    xbf = xt.astype(BF)
    idx = dispatch_indices(x, weights["_mW"], weights["_mb"])
    perm = weights["_percore"]
    corrv = weights["_corr3"]
    shared = {k: v for k, v in weights.items() if not k.startswith("_")}
    in_maps = []
    for c in range(ncores):
        lo = c * ntok
        xc = xbf[:, lo:lo + ntok]
        halo = np.zeros((D, HALO), BF)
        batch_start = not (lo >= HALO and lo % T != 0)
        if not batch_start:   # conv is causal per batch element
            halo = xbf[:, lo - HALO:lo]
        xch = np.concatenate([halo, xc], axis=1)              # [D, nt]
        m = dict(shared)
        m["x_s"] = _featmajor(xch, ntok + HALO)
        m["corr3"] = (np.ascontiguousarray(corrv) if batch_start
                      else np.zeros((3, 1024), BF))
        ide = idx[c]
        assert len(ide) <= cap, f"expert {c} count {len(ide)} > cap {cap}"
        xd = np.zeros((D, cap), BF)
        xd[:, :len(ide)] = xbf[:, ide]
        m["xd_s"] = _featmajor(xd, cap)
        for k, lst in perm.items():
            m[k] = lst[c]
        in_maps.append(m)
    return in_maps, idx


def assemble_output(results, idx, ntok=TOK, ncores=NCORE, cap=CAP):
    cols = []
    for c in range(ncores):
        o = results[c]["outT"]                                # [128, 8*ntok]
        cols.append(o.reshape(128, DC, ntok).transpose(1, 0, 2)
                    .reshape(D, ntok))
    full = np.concatenate(cols, axis=1).astype(np.float32)    # [D, tokens]
    for c in range(ncores):
        od = results[c]["outD"]                               # [128, 8*cap]
        od = od.reshape(128, DC, cap).transpose(1, 0, 2).reshape(D, cap)
        ide = idx[c]
        full[:, ide] += od[:, :len(ide)]
    return np.ascontiguousarray(full.T).reshape(B, T, D).astype(np.float32)


_CACHED = {}


def kernel(**inputs):
    x = np.asarray(inputs["x"], np.float32)
    names = ["rW", "rb", "d1W", "d1b", "d2W", "d2b", "sW_in", "sb_in",
             "sW_conv", "sb_conv", "sW_out", "sb_out", "mW", "mb",
             "eW1", "eb1", "eW2", "eb2"]
    wargs = [np.asarray(inputs[n], np.float32) for n in names]
    weights = pack_weights(*wargs)
    idx0 = dispatch_indices(x, weights["_mW"], weights["_mb"])
    need = max(len(i) for i in idx0)
    cap = max(CAP, -(-need // 128) * 128)
    if cap not in _CACHED:
        _CACHED[cap] = build_program(TOK, cap)
    nc = _CACHED[cap]
    _CACHED["last"] = (nc, cap)
    in_maps, idx = make_in_maps(x, weights, cap=cap)
    res = bass_utils.run_bass_kernel_spmd(
        nc, in_maps, core_ids=list(range(NCORE)))
    return assemble_output(res.results, idx, cap=cap)
